# revision 44
# baseline (speedup 1.0000x reference)
"""Trainium2 Bass kernel for LGCore GNN message-passing layer.

Computation (see harness reference):
  conv1 = GraphConv(curr_h, Wc, bc) * conv_w
  fused = curr_inc @ next_h
  conv2 = GraphConv(fused, Wf, bf) * topDown_w
  out   = relu(LN(0.5*(conv1+conv2)) * gamma + beta)

GraphConv is linear, so the DxD weights fold to the left of aggregation:
  res_preLN = A_hat @ (curr_h @ Wc' + curr_inc @ (next_h @ Wf')) + b'
with Wc' = 0.5*Wc*diag(conv_w), Wf' = 0.5*Wf*diag(topDown_w),
b' = 0.5*(bc*conv_w + bf*topDown_w), A_hat = diag(r_in)(A^T + I)diag(r_out).

Strategy (8 NeuronCores, SPMD):
  Launch 1: row-parallel augmented GEMM zT = [nhW ; Wc']^T @ [inc | curr_h]^T
    per core (2048 rows), contraction dim 8192+128 on partitions. inc is
    host-cast to fp8(e4m3) and multiplied against nhW split into fp8 value +
    fp8 residual via DoubleRow matmuls (2 k-chunks per instruction, 0.5
    cyc/row); curr_h stays bf16. Validated end-to-end error 6.2e-3 << 2e-2.
  Host: assemble z, scale rows by r_out -> bf16 gather source gz.
  Launch 2: dst rows permuted into 8 cores x 16 blocks of 128, bins balanced
    to exactly E/128 edges each (LPT + swap refinement, so cstar=32 with no
    gather padding). Edge rows of gz stream in via 1024-idx dma_gather calls
    (the SWDGE per-call cap; calls decoupled from block boundaries); one-hot
    matrices from is_equal(iota, dst-local id) segment-sum them via PE
    matmuls; the self-loop row block is added with an identity matmul into
    the same PSUM accumulation. With b'==0 the r_in scaling cancels inside
    LayerNorm (row-scale invariance), so the epilogue is bn_stats/bn_aggr,
    sqrt(+eps bias) on the scalar engine, reciprocal + normalize on DVE,
    relu on the scalar engine. Host inverse-permutes the 2048 dst rows.
"""

import heapq
import sys
from contextlib import ExitStack

import numpy as np

sys.path.insert(0, "/opt/trn_rl_repo")

import ml_dtypes  # noqa: E402
import concourse.bass as bass  # noqa: E402
import concourse.tile as tile  # noqa: E402
from concourse import bacc, bass_utils, mybir  # noqa: E402

F32 = mybir.dt.float32
BF16 = mybir.dt.bfloat16
F8 = mybir.dt.float8e4
I16 = mybir.dt.int16
AX_X = mybir.AxisListType.X
OP = mybir.AluOpType
ACTF = mybir.ActivationFunctionType

N, M, E, D = 16384, 8192, 524288, 128
NCORES = 8
RPC = N // NCORES            # rows per core (2048)
NBLK = RPC // 128            # dst blocks per core (16)
LN_EPS = 1e-5
INC_DT = "f8dr"              # "bf16" | "f8" | "f8dr" (DoubleRow)

_cache = {}


def _mk_bass(scratch=16384):
    return bacc.Bacc(
        "TRN2", target_bir_lowering=False, debug=False,
        enable_asserts=False, num_devices=NCORES,
        dynamic_dma_scratch_size=scratch,
    )


def build_launch1(m_dim, rpc, inc_dt):
    """zT[d, m] = sum_k incAug[k, m] * nhAug[k, d] for this core's rows."""
    nc = _mk_bass()
    KT = m_dim // 128            # inc k-chunks (64)
    GW = min(512, rpc)           # PSUM group width
    MT = rpc // GW
    idt = BF16 if inc_dt == "bf16" else F8
    incT = nc.dram_tensor("incT", [m_dim, rpc], idt, kind="ExternalInput")
    chT = nc.dram_tensor("chT", [128, rpc], BF16, kind="ExternalInput")
    nhp = nc.dram_tensor("nhp", [128, (KT + 1) * D], BF16, kind="ExternalInput")
    zT = nc.dram_tensor("zT", [128, rpc], F32, kind="ExternalOutput")
    with tile.TileContext(nc) as tc, ExitStack() as ctx:
        nh_pool = ctx.enter_context(tc.tile_pool(name="nh", bufs=1))
        inc_pool = ctx.enter_context(tc.tile_pool(name="inc", bufs=6))
        ps_pool = ctx.enter_context(tc.tile_pool(name="ps", bufs=1, space="PSUM"))
        out_pool = ctx.enter_context(tc.tile_pool(name="outt", bufs=4))
        nh_sb = nh_pool.tile([128, (KT + 1) * D], BF16)
        # staged so the first matmuls aren't gated behind one big transfer
        nc.scalar.dma_start(nh_sb[:, 0:4 * D], nhp.ap()[:, 0:4 * D])
        nc.scalar.dma_start(nh_sb[:, 4 * D:16 * D], nhp.ap()[:, 4 * D:16 * D])
        nc.scalar.dma_start(nh_sb[:, 16 * D:(KT + 1) * D],
                            nhp.ap()[:, 16 * D:(KT + 1) * D])
        ch_sb = nh_pool.tile([128, rpc], BF16)
        nc.scalar.dma_start(ch_sb[:], chT.ap())
        ps = [ps_pool.tile([128, GW], F32, name=f"psg{g}", tag=f"psg{g}")
              for g in range(MT)]
        for k in range(KT):
            it = inc_pool.tile([128, rpc], idt)
            nc.sync.dma_start(it[:], incT.ap()[k * 128:(k + 1) * 128, :])
            for g in range(MT):
                nc.tensor.matmul(
                    ps[g][:],
                    nh_sb[:, k * D:(k + 1) * D],
                    it[:, g * GW:(g + 1) * GW],
                    start=(k == 0), stop=False,
                )
        for g in range(MT):
            nc.tensor.matmul(
                ps[g][:],
                nh_sb[:, KT * D:(KT + 1) * D],
                ch_sb[:, g * GW:(g + 1) * GW],
                start=False, stop=True,
            )
        for g in range(MT):
            ot = out_pool.tile([128, GW], F32)
            if g % 2 == 0:
                nc.vector.tensor_copy(ot[:], ps[g][:])
            else:
                nc.scalar.copy(ot[:], ps[g][:])
            nc.sync.dma_start(zT.ap()[:, g * GW:(g + 1) * GW], ot[:])
    nc.compile()
    return nc


def build_launch1_dr(m_dim, rpc):
    """fp8 DoubleRow variant: inc fp8 pairs vs fp8 nh (value + residual)."""
    nc = _mk_bass()
    KT = m_dim // 128
    GW = min(512, rpc)
    MT = rpc // GW
    DR = mybir.MatmulPerfMode.DoubleRow
    incT = nc.dram_tensor("incT", [m_dim, rpc], F8, kind="ExternalInput")
    chT = nc.dram_tensor("chT", [128, rpc], BF16, kind="ExternalInput")
    nh1 = nc.dram_tensor("nh1", [128, KT * D], F8, kind="ExternalInput")
    nh2 = nc.dram_tensor("nh2", [128, KT * D], F8, kind="ExternalInput")
    wcb = nc.dram_tensor("wcb", [128, D], BF16, kind="ExternalInput")
    zT = nc.dram_tensor("zT", [128, rpc], BF16, kind="ExternalOutput")
    with tile.TileContext(nc) as tc, ExitStack() as ctx:
        nh_pool = ctx.enter_context(tc.tile_pool(name="nh", bufs=1))
        inc_pool = ctx.enter_context(tc.tile_pool(name="inc", bufs=6))
        ps_pool = ctx.enter_context(tc.tile_pool(name="ps", bufs=1, space="PSUM"))
        out_pool = ctx.enter_context(tc.tile_pool(name="outt", bufs=4))
        nh1_sb = nh_pool.tile([128, KT, D], F8)
        nc.scalar.dma_start(nh1_sb[:, 0:8, :], nh1.ap()[:, 0:8 * D])
        nc.scalar.dma_start(nh1_sb[:, 8:KT, :], nh1.ap()[:, 8 * D:KT * D])
        nh2_sb = nh_pool.tile([128, KT, D], F8)
        nc.scalar.dma_start(nh2_sb[:, 0:8, :], nh2.ap()[:, 0:8 * D])
        nc.scalar.dma_start(nh2_sb[:, 8:KT, :], nh2.ap()[:, 8 * D:KT * D])
        wcb_sb = nh_pool.tile([128, D], BF16)
        nc.scalar.dma_start(wcb_sb[:], wcb.ap())
        ch_sb = nh_pool.tile([128, rpc], BF16)
        nc.scalar.dma_start(ch_sb[:], chT.ap())
        ps = [ps_pool.tile([128, GW], F32, name=f"psg{g}", tag=f"psg{g}")
              for g in range(MT)]
        for k2 in range(KT // 2):
            it = inc_pool.tile([128, 2, rpc], F8)
            nc.sync.dma_start(
                it[:, 0, :], incT.ap()[2 * k2 * 128:(2 * k2 + 1) * 128, :])
            nc.sync.dma_start(
                it[:, 1, :], incT.ap()[(2 * k2 + 1) * 128:(2 * k2 + 2) * 128, :])
            last = k2 == KT // 2 - 1
            for g in range(MT):
                nc.tensor.matmul(
                    ps[g][:], nh1_sb[:, 2 * k2:2 * k2 + 2, :],
                    it[:, :, g * GW:(g + 1) * GW],
                    start=(k2 == 0), stop=False, perf_mode=DR,
                )
                nc.tensor.matmul(
                    ps[g][:], nh2_sb[:, 2 * k2:2 * k2 + 2, :],
                    it[:, :, g * GW:(g + 1) * GW],
                    start=False, stop=last, perf_mode=DR,
                )
            if k2 == 4:
                # curr_h @ Wc' term mid-stream: off the head (chT still
                # loading) and off the tail (accumulation ends on a cheap
                # fp8 pair instead)
                for g in range(MT):
                    nc.tensor.matmul(
                        ps[g][:], wcb_sb[:], ch_sb[:, g * GW:(g + 1) * GW],
                        start=False, stop=False,
                    )
        ot = out_pool.tile([128, rpc], BF16)
        for g in range(MT):
            if g % 2 == 0:
                nc.vector.tensor_copy(ot[:, g * GW:(g + 1) * GW], ps[g][:])
            else:
                nc.scalar.copy(ot[:, g * GW:(g + 1) * GW], ps[g][:])
        nc.sync.dma_start(zT.ap(), ot[:])
    nc.compile()
    return nc


def build_launch2(n_nodes, cstar, nblk, trivial_affine, trivial_bias):
    """Aggregation + LN + relu for this core's nblk blocks of 128 dsts.

    trivial_bias: b' == 0, so the pre-LN row scaling by r_in cancels inside
    LayerNorm (LN is scale-invariant per row) and rio/brep are not needed.
    """
    nc = _mk_bass()
    CB = cstar * 128             # padded edges per block
    EP = nblk * CB               # padded edges per core
    gz = nc.dram_tensor("gz", [n_nodes, D], BF16, kind="ExternalInput")
    idx = nc.dram_tensor("idx", [128, EP // 16], I16, kind="ExternalInput")
    dl = nc.dram_tensor("dl", [128, EP // 128], F32, kind="ExternalInput")
    ownz = nc.dram_tensor("ownz", [128, nblk * D], BF16, kind="ExternalInput")
    rio = nc.dram_tensor("rio", [128, nblk], F32, kind="ExternalInput")
    brep = nc.dram_tensor("brep", [128, D], F32, kind="ExternalInput")
    grep = nc.dram_tensor("grep", [128, D], F32, kind="ExternalInput")
    berep = nc.dram_tensor("berep", [128, D], F32, kind="ExternalInput")
    iotab = nc.dram_tensor("iotab", [128, 128], BF16, kind="ExternalInput")
    identb = nc.dram_tensor("identb", [128, 128], BF16, kind="ExternalInput")
    outp = nc.dram_tensor("outp", [128, nblk * D], BF16, kind="ExternalOutput")

    with tile.TileContext(nc) as tc, ExitStack() as ctx:
        cpool = ctx.enter_context(tc.tile_pool(name="consts", bufs=1))
        gpool = ctx.enter_context(tc.tile_pool(name="gath", bufs=14))
        spool = ctx.enter_context(tc.tile_pool(name="smat", bufs=80))
        lnp = ctx.enter_context(tc.tile_pool(name="lnp", bufs=4))
        stat = ctx.enter_context(tc.tile_pool(name="stat", bufs=8))
        opool = ctx.enter_context(tc.tile_pool(name="opool", bufs=2))
        ps_agg = ctx.enter_context(tc.tile_pool(name="psagg", bufs=2, space="PSUM"))

        def cload(handle, shape, dtype, eng=None):
            t = cpool.tile(shape, dtype, tag=handle.name)
            (eng or nc.scalar).dma_start(t[:], handle.ap())
            return t

        idx_sb = cpool.tile([128, EP // 16], I16, tag=idx.name)
        nc.sync.dma_start(idx_sb[:, 0:64], idx.ap()[:, 0:64])
        nc.sync.dma_start(idx_sb[:, 64:EP // 16], idx.ap()[:, 64:EP // 16])
        dl_sb = cload(dl, [128, EP // 128], F32)
        ownz_sb = cload(ownz, [128, nblk * D], BF16)
        if not trivial_bias:
            rio_sb = cload(rio, [128, nblk], F32)
            brep_sb = cload(brep, [128, D], F32)
        if not trivial_affine:
            grep_sb = cload(grep, [128, D], F32)
            berep_sb = cload(berep, [128, D], F32)
        iota_sb = cload(iotab, [128, 128], BF16)
        ident_sb = cload(identb, [128, 128], BF16)
        eps_sb = cpool.tile([128, 1], F32, tag="epsc")
        nc.vector.memset(eps_sb[:], LN_EPS)

        # gather calls are capped at 1024 idxs (SWDGE ring) and decoupled
        # from block boundaries: call j covers global chunks 8j..8j+7.
        GN = 8                      # chunks per gather call
        total_chunks = nblk * cstar
        gtiles = {}
        next_call = 0

        # call schedule in chunks: full GN-chunk calls, but split the final
        # call in half so the last-arriving data gates minimal tail compute
        call_sizes = [GN] * (total_chunks // GN - 1)
        call_sizes += [GN - GN // 2, GN // 2]
        call_start = [0]
        for csz in call_sizes:
            call_start.append(call_start[-1] + csz)
        chunk2call = np.repeat(np.arange(len(call_sizes)), call_sizes)

        def ensure_gathered(chunk_hi):
            nonlocal next_call
            while next_call < len(call_sizes) and call_start[next_call] <= chunk_hi:
                j = next_call
                c0, csz = call_start[j], call_sizes[j]
                n_i = csz * 128
                gt = gpool.tile([128, GN, D], BF16, name="gt")
                nc.gpsimd.dma_gather(
                    gt[:, :csz, :], gz.ap(),
                    idx_sb[:, c0 * 128 // 16:(c0 * 128 + n_i) // 16],
                    n_i, n_i, D,
                )
                gtiles[j] = gt
                next_call += 1

        for b in range(nblk):
            ensure_gathered(min(b * cstar + cstar - 1, total_chunks - 1))
            ps = ps_agg.tile([128, D], F32)
            # self-loop row block enters the accumulation via identity matmul
            nc.tensor.matmul(
                ps[:], ident_sb[:], ownz_sb[:, b * D:(b + 1) * D],
                start=True, stop=False,
            )
            for c in range(cstar):
                jc = b * cstar + c
                s = spool.tile([128, 128], BF16)
                nc.vector.tensor_scalar(
                    s[:], iota_sb[:],
                    dl_sb[:, jc: jc + 1],
                    None, op0=OP.is_equal,
                )
                cj = int(chunk2call[jc])
                nc.tensor.matmul(
                    ps[:], s[:], gtiles[cj][:, jc - call_start[cj], :],
                    start=False, stop=(c == cstar - 1),
                )
            if trivial_bias:
                # LN is row-scale invariant: skip r_in and the zero bias
                res = ps
            else:
                res = lnp.tile([128, D], F32)
                nc.vector.scalar_tensor_tensor(
                    res[:], ps[:], rio_sb[:, b:b + 1], brep_sb[:],
                    op0=OP.mult, op1=OP.add,
                )
            # LayerNorm over feature dim + affine + relu
            stats = stat.tile([128, 6], F32)
            nc.vector.bn_stats(stats[:], res[:])
            mv = stat.tile([128, 2], F32)
            nc.vector.bn_aggr(mv[:], stats[:])
            sd = stat.tile([128, 1], F32)
            nc.scalar.activation(sd[:], mv[:, 1:2], ACTF.Sqrt, bias=eps_sb[:, 0:1])
            rstd = stat.tile([128, 1], F32)
            nc.vector.reciprocal(rstd[:], sd[:])
            u = lnp.tile([128, D], F32)
            nc.vector.tensor_scalar(
                u[:], res[:], mv[:, 0:1], rstd[:],
                op0=OP.subtract, op1=OP.mult,
            )
            if not trivial_affine:
                v = lnp.tile([128, D], F32)
                nc.gpsimd.tensor_mul(v[:], u[:], grep_sb[:])
                w = lnp.tile([128, D], F32)
                nc.gpsimd.tensor_add(w[:], v[:], berep_sb[:])
            else:
                w = u
            of = opool.tile([128, D], BF16)
            nc.scalar.activation(of[:], w[:], ACTF.Relu)
            nc.sync.dma_start(outp.ap()[:, b * D:(b + 1) * D], of[:])
    nc.compile()
    return nc


def _balance_bins(dst, n_nodes, nbins):
    """Assign each dst node to one of nbins bins of exactly (n/nbins) slots,
    LPT-balancing total edge count per bin, then local-search swaps toward a
    perfectly even split (shrinks the padded chunk count). Returns
    perm[nbins, cap]."""
    cap = n_nodes // nbins
    cnt = np.bincount(dst, minlength=n_nodes)
    order = np.argsort(-cnt, kind="stable")
    heap = [(0, i) for i in range(nbins)]
    heapq.heapify(heap)
    fill = np.zeros(nbins, np.int64)
    loads = np.zeros(nbins, np.int64)
    perm = np.empty((nbins, cap), np.int64)
    for node in order:
        load, i = heapq.heappop(heap)
        perm[i, fill[i]] = node
        fill[i] += 1
        loads[i] = load + int(cnt[node])
        if fill[i] < cap:
            heapq.heappush(heap, (loads[i], i))
    assert (fill == cap).all()

    # refinement: swap nodes between heaviest/lightest bins while it helps
    tgt = int(-(-loads.max() // 128)) - 1   # try to reach one fewer chunk
    target = tgt * 128
    for _ in range(20000):
        a = int(np.argmax(loads))
        if loads[a] <= target:
            break
        b = int(np.argmin(loads))
        want = min((loads[a] - loads[b]) // 2, loads[a] - target)
        if want <= 0:
            break
        da = cnt[perm[a]]
        db = cnt[perm[b]]
        diff = da[:, None] - db[None, :]      # swap gain matrix
        good = np.where(diff > 0, np.abs(diff - want), 1 << 30)
        ia, ib = np.unravel_index(np.argmin(good), good.shape)
        if diff[ia, ib] <= 0:
            break
        perm[a][ia], perm[b][ib] = perm[b][ib], perm[a][ia]
        d = int(diff[ia, ib])
        loads[a] -= d
        loads[b] += d
    return perm


def _prep(inputs, n_nodes, m_dim, e_edges, ncores):
    """Host-side index preprocessing for launch 2."""
    src = np.asarray(inputs["edge_src"]).astype(np.int64)
    dst = np.asarray(inputs["edge_dst"]).astype(np.int64)
    out_deg = np.bincount(src, minlength=n_nodes).astype(np.float32) + 1.0
    in_deg = np.bincount(dst, minlength=n_nodes).astype(np.float32) + 1.0
    r_out = (1.0 / np.sqrt(out_deg)).astype(np.float32)
    r_in = (1.0 / np.sqrt(in_deg)).astype(np.float32)

    nblk = (n_nodes // ncores) // 128
    nbins = ncores * nblk
    perm = _balance_bins(dst, n_nodes, nbins)      # [nbins, 128]
    binid = np.empty(n_nodes, np.int64)
    plocal = np.empty(n_nodes, np.int64)
    for i in range(nbins):
        binid[perm[i]] = i
        plocal[perm[i]] = np.arange(128)

    eb = binid[dst]
    epl = plocal[dst]
    order = np.lexsort((epl, eb))
    src_s, eb_s, epl_s = src[order], eb[order], epl[order]
    counts = np.bincount(eb_s, minlength=nbins)
    cstar = max(1, int(-(-counts.max() // 128)))
    CB = cstar * 128
    starts = np.zeros(nbins + 1, np.int64)
    np.cumsum(counts, out=starts[1:])

    idx_pad = np.zeros((nbins, CB), np.int64)
    dl_pad = np.full((nbins, CB), 999.0, np.float32)
    for i in range(nbins):
        k = counts[i]
        sl = slice(starts[i], starts[i + 1])
        idx_pad[i, :k] = src_s[sl]
        dl_pad[i, :k] = epl_s[sl].astype(np.float32)
    return dict(perm=perm, r_out=r_out, r_in=r_in, cstar=cstar,
                idx_pad=idx_pad, dl_pad=dl_pad, nblk=nblk)


def _pb_layout(x_rows, perm_core, nblk):
    """rows [nblk*128, d] of x gathered by perm -> SBUF layout [128, nblk*d]."""
    d = x_rows.shape[1]
    g = x_rows[perm_core.reshape(-1)]                    # [nblk*128, d]
    return np.ascontiguousarray(
        g.reshape(nblk, 128, d).transpose(1, 0, 2).reshape(128, nblk * d))


def run(inputs, n_nodes=N, m_dim=M, e_edges=E, ncores=NCORES,
        runner=None, collect=None):
    """Full pipeline. runner(nc, in_maps) -> list of per-core output dicts."""
    if runner is None:
        def runner(nc, in_maps):
            r = bass_utils.run_bass_kernel_spmd(nc, in_maps, list(range(ncores)))
            return r.results
    rpc = n_nodes // ncores
    curr_h = np.asarray(inputs["curr_h"], np.float32)
    next_h = np.asarray(inputs["next_h"], np.float32)
    inc = np.asarray(inputs["curr_inc"], np.float32)
    KT = m_dim // 128

    conv_w = np.asarray(inputs["conv_w"], np.float32)
    td_w = np.asarray(inputs["topDown_w"], np.float32)
    Wc = np.asarray(inputs["Wc"], np.float32)
    Wf = np.asarray(inputs["Wf"], np.float32)
    bc = np.asarray(inputs["bc"], np.float32)
    bf = np.asarray(inputs["bf"], np.float32)
    gamma = np.asarray(inputs["gamma"], np.float32)
    beta = np.asarray(inputs["beta"], np.float32)
    wcp = 0.5 * Wc * conv_w[None, :]
    wfp = 0.5 * Wf * td_w[None, :]
    bprime = 0.5 * (bc * conv_w + bf * td_w)
    trivial_affine = bool((gamma == 1.0).all() and (beta == 0.0).all())

    # launch 1: zT = [next_h@Wf' ; Wc']^T @ [inc | curr_h]^T
    nhW = next_h @ wfp                                   # [m_dim, D]
    nhAug = np.concatenate([nhW, wcp], axis=0)           # [(KT+1)*128, D]
    nhp = np.ascontiguousarray(
        nhAug.reshape(KT + 1, 128, D).transpose(1, 0, 2)
        .reshape(128, (KT + 1) * D)).astype(ml_dtypes.bfloat16)
    inc_np_dt = ml_dtypes.bfloat16 if INC_DT == "bf16" else ml_dtypes.float8_e4m3

    key1 = ("l1", m_dim, rpc, INC_DT)
    if key1 not in _cache:
        _cache[key1] = (build_launch1_dr(m_dim, rpc) if INC_DT == "f8dr"
                        else build_launch1(m_dim, rpc, INC_DT))
    nc1 = _cache[key1]
    if INC_DT == "f8dr":
        nh1f = nhAug[:m_dim].astype(ml_dtypes.float8_e4m3)
        nh2f = (nhAug[:m_dim] - nh1f.astype(np.float32)).astype(
            ml_dtypes.float8_e4m3)
        pk = lambda a: np.ascontiguousarray(
            a.reshape(KT, 128, D).transpose(1, 0, 2).reshape(128, KT * D))
        nh1p, nh2p = pk(nh1f), pk(nh2f)
        wcb = wcp.astype(ml_dtypes.bfloat16)
    in_maps1 = []
    for c in range(ncores):
        incT = np.ascontiguousarray(
            inc[c * rpc:(c + 1) * rpc].T).astype(inc_np_dt)
        chT = np.ascontiguousarray(
            curr_h[c * rpc:(c + 1) * rpc].T).astype(ml_dtypes.bfloat16)
        if INC_DT == "f8dr":
            in_maps1.append({"incT": incT, "chT": chT,
                             "nh1": nh1p, "nh2": nh2p, "wcb": wcb})
        else:
            in_maps1.append({"incT": incT, "chT": chT, "nhp": nhp})
    res1 = runner(nc1, in_maps1)
    z = np.concatenate(
        [np.asarray(res1[c]["zT"]).astype(np.float32).T for c in range(ncores)],
        axis=0)
    if collect is not None:
        collect["z"] = z

    pp = _prep(inputs, n_nodes, m_dim, e_edges, ncores)
    cstar, nblk = pp["cstar"], pp["nblk"]
    gz = (z * pp["r_out"][:, None]).astype(ml_dtypes.bfloat16)

    rep = lambda v: np.ascontiguousarray(
        np.tile(v[None, :], (128, 1)).astype(np.float32))
    iotab = np.tile(np.arange(128, dtype=np.float32)[None, :],
                    (128, 1)).astype(ml_dtypes.bfloat16)
    identb = np.eye(128, dtype=np.float32).astype(ml_dtypes.bfloat16)

    trivial_bias = bool((bprime == 0.0).all())
    key2 = ("l2", n_nodes, cstar, nblk, trivial_affine, trivial_bias)
    if key2 not in _cache:
        _cache[key2] = build_launch2(n_nodes, cstar, nblk, trivial_affine,
                                     trivial_bias)
    nc2 = _cache[key2]

    in_maps2 = []
    for c in range(ncores):
        perm_c = pp["perm"][c * nblk:(c + 1) * nblk]     # [nblk, 128]
        ep = nblk * cstar * 128
        idx_core = pp["idx_pad"][c * nblk:(c + 1) * nblk].reshape(ep)
        dl_core = pp["dl_pad"][c * nblk:(c + 1) * nblk].reshape(ep)
        pc_flat = perm_c.reshape(-1)
        in_maps2.append({
            "gz": gz,
            "idx": np.ascontiguousarray(np.tile(
                idx_core.reshape(-1, 16).T.astype(np.int16), (8, 1))),
            "dl": np.ascontiguousarray(dl_core.reshape(-1, 128).T),
            "ownz": _pb_layout(gz, perm_c, nblk),
            "rio": np.ascontiguousarray(
                pp["r_in"][pc_flat].reshape(nblk, 128).T),
            "brep": rep(bprime), "grep": rep(gamma), "berep": rep(beta),
            "iotab": iotab, "identb": identb,
        })
    res2 = runner(nc2, in_maps2)
    out = np.empty((n_nodes, D), np.float32)
    for c in range(ncores):
        perm_c = pp["perm"][c * nblk:(c + 1) * nblk].reshape(-1)
        oc = np.asarray(res2[c]["outp"]).astype(np.float32)  # [128, nblk*D]
        out[perm_c] = oc.reshape(128, nblk, D).transpose(1, 0, 2).reshape(-1, D)
    return out


def kernel(**inputs):
    out = run(inputs)
    return out


# revision 52
# speedup vs baseline: 1.0515x; 1.0515x over previous
"""Trainium2 Bass kernel for LGCore GNN message-passing layer.

Computation (see harness reference):
  conv1 = GraphConv(curr_h, Wc, bc) * conv_w
  fused = curr_inc @ next_h
  conv2 = GraphConv(fused, Wf, bf) * topDown_w
  out   = relu(LN(0.5*(conv1+conv2)) * gamma + beta)

GraphConv is linear, so the DxD weights fold to the left of aggregation:
  res_preLN = A_hat @ (curr_h @ Wc' + curr_inc @ (next_h @ Wf')) + b'
with Wc' = 0.5*Wc*diag(conv_w), Wf' = 0.5*Wf*diag(topDown_w),
b' = 0.5*(bc*conv_w + bf*topDown_w), A_hat = diag(r_in)(A^T + I)diag(r_out).

Strategy (8 NeuronCores, SPMD):
  Launch 1: row-parallel augmented GEMM zT = [nhW ; Wc']^T @ [inc | curr_h]^T
    per core (2048 rows), contraction dim 8192+128 on partitions. inc is
    host-cast to fp8(e4m3) and multiplied against nhW split into fp8 value +
    fp8 residual via DoubleRow matmuls (2 k-chunks per instruction, 0.5
    cyc/row); curr_h stays bf16. Validated end-to-end error 6.2e-3 << 2e-2.
  Host: assemble z, scale rows by r_out -> bf16 gather source gz.
  Launch 2: dst rows permuted into 8 cores x 16 blocks of 128, bins balanced
    to exactly E/128 edges each (LPT + swap refinement, so cstar=32 with no
    gather padding). Edge rows of gz stream in via 1024-idx dma_gather calls
    (the SWDGE per-call cap; calls decoupled from block boundaries); one-hot
    matrices from is_equal(iota, dst-local id) segment-sum them via PE
    matmuls; the self-loop row block is added with an identity matmul into
    the same PSUM accumulation. With b'==0 the r_in scaling cancels inside
    LayerNorm (row-scale invariance), so the epilogue is bn_stats/bn_aggr,
    sqrt(+eps bias) on the scalar engine, reciprocal + normalize on DVE,
    relu on the scalar engine. Host inverse-permutes the 2048 dst rows.
"""

import heapq
import sys
from contextlib import ExitStack

import numpy as np

sys.path.insert(0, "/opt/trn_rl_repo")

import ml_dtypes  # noqa: E402
import concourse.bass as bass  # noqa: E402
import concourse.tile as tile  # noqa: E402
from concourse import bacc, bass_utils, mybir  # noqa: E402

F32 = mybir.dt.float32
BF16 = mybir.dt.bfloat16
F8 = mybir.dt.float8e4
I16 = mybir.dt.int16
AX_X = mybir.AxisListType.X
OP = mybir.AluOpType
ACTF = mybir.ActivationFunctionType

N, M, E, D = 16384, 8192, 524288, 128
NCORES = 8
RPC = N // NCORES            # rows per core (2048)
NBLK = RPC // 128            # dst blocks per core (16)
LN_EPS = 1e-5
INC_DT = "f8dr"              # "bf16" | "f8" | "f8dr" (DoubleRow)

_cache = {}


def _mk_bass(scratch=16384):
    return bacc.Bacc(
        "TRN2", target_bir_lowering=False, debug=False,
        enable_asserts=False, num_devices=NCORES,
        dynamic_dma_scratch_size=scratch,
    )


def build_launch1(m_dim, rpc, inc_dt):
    """zT[d, m] = sum_k incAug[k, m] * nhAug[k, d] for this core's rows."""
    nc = _mk_bass()
    KT = m_dim // 128            # inc k-chunks (64)
    GW = min(512, rpc)           # PSUM group width
    MT = rpc // GW
    idt = BF16 if inc_dt == "bf16" else F8
    incT = nc.dram_tensor("incT", [m_dim, rpc], idt, kind="ExternalInput")
    chT = nc.dram_tensor("chT", [128, rpc], BF16, kind="ExternalInput")
    nhp = nc.dram_tensor("nhp", [128, (KT + 1) * D], BF16, kind="ExternalInput")
    zT = nc.dram_tensor("zT", [128, rpc], F32, kind="ExternalOutput")
    with tile.TileContext(nc) as tc, ExitStack() as ctx:
        nh_pool = ctx.enter_context(tc.tile_pool(name="nh", bufs=1))
        inc_pool = ctx.enter_context(tc.tile_pool(name="inc", bufs=6))
        ps_pool = ctx.enter_context(tc.tile_pool(name="ps", bufs=1, space="PSUM"))
        out_pool = ctx.enter_context(tc.tile_pool(name="outt", bufs=4))
        nh_sb = nh_pool.tile([128, (KT + 1) * D], BF16)
        # staged so the first matmuls aren't gated behind one big transfer
        nc.scalar.dma_start(nh_sb[:, 0:4 * D], nhp.ap()[:, 0:4 * D])
        nc.scalar.dma_start(nh_sb[:, 4 * D:16 * D], nhp.ap()[:, 4 * D:16 * D])
        nc.scalar.dma_start(nh_sb[:, 16 * D:(KT + 1) * D],
                            nhp.ap()[:, 16 * D:(KT + 1) * D])
        ch_sb = nh_pool.tile([128, rpc], BF16)
        nc.scalar.dma_start(ch_sb[:], chT.ap())
        ps = [ps_pool.tile([128, GW], F32, name=f"psg{g}", tag=f"psg{g}")
              for g in range(MT)]
        for k in range(KT):
            it = inc_pool.tile([128, rpc], idt)
            nc.sync.dma_start(it[:], incT.ap()[k * 128:(k + 1) * 128, :])
            for g in range(MT):
                nc.tensor.matmul(
                    ps[g][:],
                    nh_sb[:, k * D:(k + 1) * D],
                    it[:, g * GW:(g + 1) * GW],
                    start=(k == 0), stop=False,
                )
        for g in range(MT):
            nc.tensor.matmul(
                ps[g][:],
                nh_sb[:, KT * D:(KT + 1) * D],
                ch_sb[:, g * GW:(g + 1) * GW],
                start=False, stop=True,
            )
        for g in range(MT):
            ot = out_pool.tile([128, GW], F32)
            if g % 2 == 0:
                nc.vector.tensor_copy(ot[:], ps[g][:])
            else:
                nc.scalar.copy(ot[:], ps[g][:])
            nc.sync.dma_start(zT.ap()[:, g * GW:(g + 1) * GW], ot[:])
    nc.compile()
    return nc


def build_launch1_dr(m_dim, rpc):
    """fp8 DoubleRow variant: inc fp8 pairs vs fp8 nh (value + residual)."""
    nc = _mk_bass()
    KT = m_dim // 128
    GW = min(512, rpc)
    MT = rpc // GW
    DR = mybir.MatmulPerfMode.DoubleRow
    incT = nc.dram_tensor("incT", [m_dim, rpc], F8, kind="ExternalInput")
    chT = nc.dram_tensor("chT", [128, rpc], BF16, kind="ExternalInput")
    nh1 = nc.dram_tensor("nh1", [128, KT * D], F8, kind="ExternalInput")
    nh2 = nc.dram_tensor("nh2", [128, KT * D], F8, kind="ExternalInput")
    wcb = nc.dram_tensor("wcb", [128, D], BF16, kind="ExternalInput")
    zT = nc.dram_tensor("zT", [128, rpc], BF16, kind="ExternalOutput")
    with tile.TileContext(nc) as tc, ExitStack() as ctx:
        nh_pool = ctx.enter_context(tc.tile_pool(name="nh", bufs=1))
        inc_pool = ctx.enter_context(tc.tile_pool(name="inc", bufs=6))
        ps_pool = ctx.enter_context(tc.tile_pool(name="ps", bufs=1, space="PSUM"))
        out_pool = ctx.enter_context(tc.tile_pool(name="outt", bufs=4))
        nh1_sb = nh_pool.tile([128, KT, D], F8)
        nc.scalar.dma_start(nh1_sb[:, 0:8, :], nh1.ap()[:, 0:8 * D])
        nc.scalar.dma_start(nh1_sb[:, 8:KT, :], nh1.ap()[:, 8 * D:KT * D])
        nh2_sb = nh_pool.tile([128, KT, D], F8)
        nc.scalar.dma_start(nh2_sb[:, 0:8, :], nh2.ap()[:, 0:8 * D])
        nc.scalar.dma_start(nh2_sb[:, 8:KT, :], nh2.ap()[:, 8 * D:KT * D])
        wcb_sb = nh_pool.tile([128, D], BF16)
        nc.scalar.dma_start(wcb_sb[:], wcb.ap())
        ch_sb = nh_pool.tile([128, rpc], BF16)
        nc.scalar.dma_start(ch_sb[:], chT.ap())
        ps = [ps_pool.tile([128, GW], F32, name=f"psg{g}", tag=f"psg{g}")
              for g in range(MT)]
        for k2 in range(KT // 2):
            it = inc_pool.tile([128, 2, rpc], F8)
            nc.sync.dma_start(
                it[:, 0, :], incT.ap()[2 * k2 * 128:(2 * k2 + 1) * 128, :])
            nc.sync.dma_start(
                it[:, 1, :], incT.ap()[(2 * k2 + 1) * 128:(2 * k2 + 2) * 128, :])
            last = k2 == KT // 2 - 1
            for g in range(MT):
                nc.tensor.matmul(
                    ps[g][:], nh1_sb[:, 2 * k2:2 * k2 + 2, :],
                    it[:, :, g * GW:(g + 1) * GW],
                    start=(k2 == 0), stop=False, perf_mode=DR,
                )
                nc.tensor.matmul(
                    ps[g][:], nh2_sb[:, 2 * k2:2 * k2 + 2, :],
                    it[:, :, g * GW:(g + 1) * GW],
                    start=False, stop=last, perf_mode=DR,
                )
            if k2 == 4:
                # curr_h @ Wc' term mid-stream: off the head (chT still
                # loading) and off the tail (accumulation ends on a cheap
                # fp8 pair instead)
                for g in range(MT):
                    nc.tensor.matmul(
                        ps[g][:], wcb_sb[:], ch_sb[:, g * GW:(g + 1) * GW],
                        start=False, stop=False,
                    )
        ot = out_pool.tile([128, rpc], BF16)
        for g in range(MT):
            if g % 2 == 0:
                nc.vector.tensor_copy(ot[:, g * GW:(g + 1) * GW], ps[g][:])
            else:
                nc.scalar.copy(ot[:, g * GW:(g + 1) * GW], ps[g][:])
        nc.sync.dma_start(zT.ap(), ot[:])
    nc.compile()
    return nc


def build_launch2(n_nodes, layer_cols, nblk, trivial_affine, trivial_bias):
    """Aggregation + LN + relu for this core's nblk blocks of 128 dsts.

    layer_cols[k] = chunk count of one-hot layer k per block: each gathered
    slot holds a distinct (block, src) row; layer k scatters every slot's
    k-th destination (999 = none). Layer 0 spans all cstar gathered chunks.
    trivial_bias: b' == 0, so the pre-LN row scaling by r_in cancels inside
    LayerNorm (LN is scale-invariant per row) and rio/brep are not needed.
    """
    nc = _mk_bass()
    cstar = layer_cols[0]
    CT = int(sum(layer_cols))
    offs = [0]
    for ck in layer_cols:
        offs.append(offs[-1] + ck)
    CB = cstar * 128             # gathered slots per block
    EP = nblk * CB               # gathered slots per core
    gz = nc.dram_tensor("gz", [n_nodes, D], BF16, kind="ExternalInput")
    idx = nc.dram_tensor("idx", [128, EP // 16], I16, kind="ExternalInput")
    dl = nc.dram_tensor("dl", [128, nblk * CT], F32, kind="ExternalInput")
    ownz = nc.dram_tensor("ownz", [128, nblk * D], BF16, kind="ExternalInput")
    rio = nc.dram_tensor("rio", [128, nblk], F32, kind="ExternalInput")
    brep = nc.dram_tensor("brep", [128, D], F32, kind="ExternalInput")
    grep = nc.dram_tensor("grep", [128, D], F32, kind="ExternalInput")
    berep = nc.dram_tensor("berep", [128, D], F32, kind="ExternalInput")
    iotab = nc.dram_tensor("iotab", [128, 128], BF16, kind="ExternalInput")
    identb = nc.dram_tensor("identb", [128, 128], BF16, kind="ExternalInput")
    outp = nc.dram_tensor("outp", [128, nblk * D], BF16, kind="ExternalOutput")

    with tile.TileContext(nc) as tc, ExitStack() as ctx:
        cpool = ctx.enter_context(tc.tile_pool(name="consts", bufs=1))
        gpool = ctx.enter_context(tc.tile_pool(name="gath", bufs=14))
        spool = ctx.enter_context(tc.tile_pool(name="smat", bufs=80))
        lnp = ctx.enter_context(tc.tile_pool(name="lnp", bufs=4))
        stat = ctx.enter_context(tc.tile_pool(name="stat", bufs=8))
        opool = ctx.enter_context(tc.tile_pool(name="opool", bufs=2))
        ps_agg = ctx.enter_context(tc.tile_pool(name="psagg", bufs=2, space="PSUM"))

        def cload(handle, shape, dtype, eng=None):
            t = cpool.tile(shape, dtype, tag=handle.name)
            (eng or nc.scalar).dma_start(t[:], handle.ap())
            return t

        idx_sb = cpool.tile([128, EP // 16], I16, tag=idx.name)
        nc.sync.dma_start(idx_sb[:, 0:64], idx.ap()[:, 0:64])
        nc.sync.dma_start(idx_sb[:, 64:EP // 16], idx.ap()[:, 64:EP // 16])
        dl_sb = cload(dl, [128, nblk * CT], F32)
        ownz_sb = cload(ownz, [128, nblk * D], BF16)
        if not trivial_bias:
            rio_sb = cload(rio, [128, nblk], F32)
            brep_sb = cload(brep, [128, D], F32)
        if not trivial_affine:
            grep_sb = cload(grep, [128, D], F32)
            berep_sb = cload(berep, [128, D], F32)
        iota_sb = cload(iotab, [128, 128], BF16)
        ident_sb = cload(identb, [128, 128], BF16)
        eps_sb = cpool.tile([128, 1], F32, tag="epsc")
        nc.vector.memset(eps_sb[:], LN_EPS)

        # gather calls are capped at 1024 idxs (SWDGE ring) and decoupled
        # from block boundaries: call j covers global chunks 8j..8j+7.
        GN = 8                      # chunks per gather call
        total_chunks = nblk * cstar
        gtiles = {}
        next_call = 0

        # call schedule in chunks: full GN-chunk calls, but split the final
        # call in half so the last-arriving data gates minimal tail compute
        call_sizes = [GN] * (total_chunks // GN - 1)
        call_sizes += [GN - GN // 2, GN // 2]
        call_start = [0]
        for csz in call_sizes:
            call_start.append(call_start[-1] + csz)
        chunk2call = np.repeat(np.arange(len(call_sizes)), call_sizes)

        def ensure_gathered(chunk_hi):
            nonlocal next_call
            while next_call < len(call_sizes) and call_start[next_call] <= chunk_hi:
                j = next_call
                c0, csz = call_start[j], call_sizes[j]
                n_i = csz * 128
                gt = gpool.tile([128, GN, D], BF16, name="gt")
                nc.gpsimd.dma_gather(
                    gt[:, :csz, :], gz.ap(),
                    idx_sb[:, c0 * 128 // 16:(c0 * 128 + n_i) // 16],
                    n_i, n_i, D,
                )
                gtiles[j] = gt
                next_call += 1

        for b in range(nblk):
            ensure_gathered(min(b * cstar + cstar - 1, total_chunks - 1))
            ps = ps_agg.tile([128, D], F32)
            # self-loop row block enters the accumulation via identity matmul
            nc.tensor.matmul(
                ps[:], ident_sb[:], ownz_sb[:, b * D:(b + 1) * D],
                start=True, stop=False,
            )
            passes = [(k, c) for k in range(len(layer_cols))
                      for c in range(layer_cols[k])]
            for pi, (k, c) in enumerate(passes):
                jc = b * cstar + c            # gathered chunk (shared by layers)
                col = b * CT + offs[k] + c    # this layer's dst-id column
                s = spool.tile([128, 128], BF16)
                nc.vector.tensor_scalar(
                    s[:], iota_sb[:],
                    dl_sb[:, col: col + 1],
                    None, op0=OP.is_equal,
                )
                cj = int(chunk2call[jc])
                nc.tensor.matmul(
                    ps[:], s[:], gtiles[cj][:, jc - call_start[cj], :],
                    start=False, stop=(pi == len(passes) - 1),
                )
            if trivial_bias:
                # LN is row-scale invariant: skip r_in and the zero bias
                res = ps
            else:
                res = lnp.tile([128, D], F32)
                nc.vector.scalar_tensor_tensor(
                    res[:], ps[:], rio_sb[:, b:b + 1], brep_sb[:],
                    op0=OP.mult, op1=OP.add,
                )
            # LayerNorm over feature dim + affine + relu
            stats = stat.tile([128, 6], F32)
            nc.vector.bn_stats(stats[:], res[:])
            mv = stat.tile([128, 2], F32)
            nc.vector.bn_aggr(mv[:], stats[:])
            sd = stat.tile([128, 1], F32)
            nc.scalar.activation(sd[:], mv[:, 1:2], ACTF.Sqrt, bias=eps_sb[:, 0:1])
            rstd = stat.tile([128, 1], F32)
            nc.vector.reciprocal(rstd[:], sd[:])
            u = lnp.tile([128, D], F32)
            nc.vector.tensor_scalar(
                u[:], res[:], mv[:, 0:1], rstd[:],
                op0=OP.subtract, op1=OP.mult,
            )
            if not trivial_affine:
                v = lnp.tile([128, D], F32)
                nc.gpsimd.tensor_mul(v[:], u[:], grep_sb[:])
                w = lnp.tile([128, D], F32)
                nc.gpsimd.tensor_add(w[:], v[:], berep_sb[:])
            else:
                w = u
            of = opool.tile([128, D], BF16)
            nc.scalar.activation(of[:], w[:], ACTF.Relu)
            nc.sync.dma_start(outp.ap()[:, b * D:(b + 1) * D], of[:])
    nc.compile()
    return nc


def _balance_bins(dst, n_nodes, nbins):
    """Assign each dst node to one of nbins bins of exactly (n/nbins) slots,
    LPT-balancing total edge count per bin, then local-search swaps toward a
    perfectly even split (shrinks the padded chunk count). Returns
    perm[nbins, cap]."""
    cap = n_nodes // nbins
    cnt = np.bincount(dst, minlength=n_nodes)
    order = np.argsort(-cnt, kind="stable")
    heap = [(0, i) for i in range(nbins)]
    heapq.heapify(heap)
    fill = np.zeros(nbins, np.int64)
    loads = np.zeros(nbins, np.int64)
    perm = np.empty((nbins, cap), np.int64)
    for node in order:
        load, i = heapq.heappop(heap)
        perm[i, fill[i]] = node
        fill[i] += 1
        loads[i] = load + int(cnt[node])
        if fill[i] < cap:
            heapq.heappush(heap, (loads[i], i))
    assert (fill == cap).all()

    # refinement: swap nodes between heaviest/lightest bins while it helps
    tgt = int(-(-loads.max() // 128)) - 1   # try to reach one fewer chunk
    target = tgt * 128
    for _ in range(20000):
        a = int(np.argmax(loads))
        if loads[a] <= target:
            break
        b = int(np.argmin(loads))
        want = min((loads[a] - loads[b]) // 2, loads[a] - target)
        if want <= 0:
            break
        da = cnt[perm[a]]
        db = cnt[perm[b]]
        diff = da[:, None] - db[None, :]      # swap gain matrix
        good = np.where(diff > 0, np.abs(diff - want), 1 << 30)
        ia, ib = np.unravel_index(np.argmin(good), good.shape)
        if diff[ia, ib] <= 0:
            break
        perm[a][ia], perm[b][ib] = perm[b][ib], perm[a][ia]
        d = int(diff[ia, ib])
        loads[a] -= d
        loads[b] += d
    return perm


def _prep(inputs, n_nodes, m_dim, e_edges, ncores):
    """Host-side index preprocessing for launch 2."""
    src = np.asarray(inputs["edge_src"]).astype(np.int64)
    dst = np.asarray(inputs["edge_dst"]).astype(np.int64)
    out_deg = np.bincount(src, minlength=n_nodes).astype(np.float32) + 1.0
    in_deg = np.bincount(dst, minlength=n_nodes).astype(np.float32) + 1.0
    r_out = (1.0 / np.sqrt(out_deg)).astype(np.float32)
    r_in = (1.0 / np.sqrt(in_deg)).astype(np.float32)

    nblk = (n_nodes // ncores) // 128
    nbins = ncores * nblk
    perm = _balance_bins(dst, n_nodes, nbins)      # [nbins, 128]
    binid = np.empty(n_nodes, np.int64)
    plocal = np.empty(n_nodes, np.int64)
    for i in range(nbins):
        binid[perm[i]] = i
        plocal[perm[i]] = np.arange(128)

    # deduplicate (bin, src) pairs: gather each distinct src once per bin,
    # scatter to its 1..L destinations via L one-hot layers
    eb = binid[dst]
    epl = plocal[dst]
    order = np.lexsort((src, eb))
    src_s, eb_s, epl_s = src[order], eb[order], epl[order]
    key = eb_s * (n_nodes + 1) + src_s
    new = np.ones(len(key), bool)
    new[1:] = key[1:] != key[:-1]
    gid = np.cumsum(new) - 1                       # slot id per edge
    gstart = np.flatnonzero(new)
    gcount = np.diff(np.append(gstart, len(key)))  # edges per slot
    rank = np.arange(len(key)) - gstart[gid]       # 0-based layer per edge
    gbin = eb_s[gstart]
    gsrc = src_s[gstart]
    # slot positions within each bin, multiplicity-descending
    sorder = np.lexsort((-gcount, gbin))
    nslot_bin = np.bincount(gbin, minlength=nbins)
    bstart = np.zeros(nbins + 1, np.int64)
    np.cumsum(nslot_bin, out=bstart[1:])
    posw = np.arange(len(sorder)) - bstart[gbin[sorder]]
    slotpos = np.empty(len(sorder), np.int64)
    slotpos[sorder] = posw
    L = int(gcount.max())
    layer_cols = []
    for k in range(1, L + 1):
        mk = np.bincount(gbin[gcount >= k], minlength=nbins).max()
        layer_cols.append(max(1, int(-(-int(mk) // 128))))
    C1 = layer_cols[0]
    idx_pad = np.zeros((nbins, C1 * 128), np.int64)
    idx_pad[gbin, slotpos] = gsrc
    CT = int(sum(layer_cols))
    offs = np.cumsum([0] + layer_cols)
    dl_pad = np.full((nbins, CT * 128), 999.0, np.float32)
    epos = slotpos[gid]
    ecol = offs[rank] * 128 + epos
    dl_pad[eb_s, ecol] = epl_s.astype(np.float32)
    return dict(perm=perm, r_out=r_out, r_in=r_in, layer_cols=layer_cols,
                idx_pad=idx_pad, dl_pad=dl_pad, nblk=nblk)


def _pb_layout(x_rows, perm_core, nblk):
    """rows [nblk*128, d] of x gathered by perm -> SBUF layout [128, nblk*d]."""
    d = x_rows.shape[1]
    g = x_rows[perm_core.reshape(-1)]                    # [nblk*128, d]
    return np.ascontiguousarray(
        g.reshape(nblk, 128, d).transpose(1, 0, 2).reshape(128, nblk * d))


def run(inputs, n_nodes=N, m_dim=M, e_edges=E, ncores=NCORES,
        runner=None, collect=None):
    """Full pipeline. runner(nc, in_maps) -> list of per-core output dicts."""
    if runner is None:
        def runner(nc, in_maps):
            r = bass_utils.run_bass_kernel_spmd(nc, in_maps, list(range(ncores)))
            return r.results
    rpc = n_nodes // ncores
    curr_h = np.asarray(inputs["curr_h"], np.float32)
    next_h = np.asarray(inputs["next_h"], np.float32)
    inc = np.asarray(inputs["curr_inc"], np.float32)
    KT = m_dim // 128

    conv_w = np.asarray(inputs["conv_w"], np.float32)
    td_w = np.asarray(inputs["topDown_w"], np.float32)
    Wc = np.asarray(inputs["Wc"], np.float32)
    Wf = np.asarray(inputs["Wf"], np.float32)
    bc = np.asarray(inputs["bc"], np.float32)
    bf = np.asarray(inputs["bf"], np.float32)
    gamma = np.asarray(inputs["gamma"], np.float32)
    beta = np.asarray(inputs["beta"], np.float32)
    wcp = 0.5 * Wc * conv_w[None, :]
    wfp = 0.5 * Wf * td_w[None, :]
    bprime = 0.5 * (bc * conv_w + bf * td_w)
    trivial_affine = bool((gamma == 1.0).all() and (beta == 0.0).all())

    # launch 1: zT = [next_h@Wf' ; Wc']^T @ [inc | curr_h]^T
    nhW = next_h @ wfp                                   # [m_dim, D]
    nhAug = np.concatenate([nhW, wcp], axis=0)           # [(KT+1)*128, D]
    nhp = np.ascontiguousarray(
        nhAug.reshape(KT + 1, 128, D).transpose(1, 0, 2)
        .reshape(128, (KT + 1) * D)).astype(ml_dtypes.bfloat16)
    inc_np_dt = ml_dtypes.bfloat16 if INC_DT == "bf16" else ml_dtypes.float8_e4m3

    key1 = ("l1", m_dim, rpc, INC_DT)
    if key1 not in _cache:
        _cache[key1] = (build_launch1_dr(m_dim, rpc) if INC_DT == "f8dr"
                        else build_launch1(m_dim, rpc, INC_DT))
    nc1 = _cache[key1]
    if INC_DT == "f8dr":
        nh1f = nhAug[:m_dim].astype(ml_dtypes.float8_e4m3)
        nh2f = (nhAug[:m_dim] - nh1f.astype(np.float32)).astype(
            ml_dtypes.float8_e4m3)
        pk = lambda a: np.ascontiguousarray(
            a.reshape(KT, 128, D).transpose(1, 0, 2).reshape(128, KT * D))
        nh1p, nh2p = pk(nh1f), pk(nh2f)
        wcb = wcp.astype(ml_dtypes.bfloat16)
    in_maps1 = []
    for c in range(ncores):
        incT = np.ascontiguousarray(
            inc[c * rpc:(c + 1) * rpc].T).astype(inc_np_dt)
        chT = np.ascontiguousarray(
            curr_h[c * rpc:(c + 1) * rpc].T).astype(ml_dtypes.bfloat16)
        if INC_DT == "f8dr":
            in_maps1.append({"incT": incT, "chT": chT,
                             "nh1": nh1p, "nh2": nh2p, "wcb": wcb})
        else:
            in_maps1.append({"incT": incT, "chT": chT, "nhp": nhp})
    res1 = runner(nc1, in_maps1)
    z = np.concatenate(
        [np.asarray(res1[c]["zT"]).astype(np.float32).T for c in range(ncores)],
        axis=0)
    if collect is not None:
        collect["z"] = z

    pp = _prep(inputs, n_nodes, m_dim, e_edges, ncores)
    layer_cols, nblk = pp["layer_cols"], pp["nblk"]
    cstar = layer_cols[0]
    CT = int(sum(layer_cols))
    gz = (z * pp["r_out"][:, None]).astype(ml_dtypes.bfloat16)

    rep = lambda v: np.ascontiguousarray(
        np.tile(v[None, :], (128, 1)).astype(np.float32))
    iotab = np.tile(np.arange(128, dtype=np.float32)[None, :],
                    (128, 1)).astype(ml_dtypes.bfloat16)
    identb = np.eye(128, dtype=np.float32).astype(ml_dtypes.bfloat16)

    trivial_bias = bool((bprime == 0.0).all())
    key2 = ("l2", n_nodes, tuple(layer_cols), nblk, trivial_affine,
            trivial_bias)
    if key2 not in _cache:
        _cache[key2] = build_launch2(n_nodes, layer_cols, nblk,
                                     trivial_affine, trivial_bias)
    nc2 = _cache[key2]

    in_maps2 = []
    for c in range(ncores):
        perm_c = pp["perm"][c * nblk:(c + 1) * nblk]     # [nblk, 128]
        ep = nblk * cstar * 128
        idx_core = pp["idx_pad"][c * nblk:(c + 1) * nblk].reshape(ep)
        dl_core = pp["dl_pad"][c * nblk:(c + 1) * nblk].reshape(nblk * CT * 128)
        pc_flat = perm_c.reshape(-1)
        in_maps2.append({
            "gz": gz,
            "idx": np.ascontiguousarray(np.tile(
                idx_core.reshape(-1, 16).T.astype(np.int16), (8, 1))),
            "dl": np.ascontiguousarray(dl_core.reshape(-1, 128).T),
            "ownz": _pb_layout(gz, perm_c, nblk),
            "rio": np.ascontiguousarray(
                pp["r_in"][pc_flat].reshape(nblk, 128).T),
            "brep": rep(bprime), "grep": rep(gamma), "berep": rep(beta),
            "iotab": iotab, "identb": identb,
        })
    res2 = runner(nc2, in_maps2)
    out = np.empty((n_nodes, D), np.float32)
    for c in range(ncores):
        perm_c = pp["perm"][c * nblk:(c + 1) * nblk].reshape(-1)
        oc = np.asarray(res2[c]["outp"]).astype(np.float32)  # [128, nblk*D]
        out[perm_c] = oc.reshape(128, nblk, D).transpose(1, 0, 2).reshape(-1, D)
    return out


def kernel(**inputs):
    out = run(inputs)
    return out
